# revision 19
# baseline (speedup 1.0000x reference)
"""MPI compositing + homography warp kernel for Trainium2 (8 NeuronCores).

For each of P=32 fronto-parallel planes and S=4 source images: composite
per-plane channels (net transmittance T, accumulated-over acc, full-over
bro, source image src -> 10 channels), then bilinear-warp each (plane, src)
channel stack by a plane/source-dependent homography. Output (P, S, 10, H, W).

Structure exploited: the target->source homography here has identity
rotation and shared intrinsics, so sample coordinate ix depends only on x
and iy only on y.  The bilinear gather (zero padding) then factorizes
EXACTLY into two small banded matrices applied left/right:

    warped = Wy @ S @ Wx^T        per (plane, src, channel)

with per-tap validity folded into the weights.  Wy/Wx are built on the host
from the pose inputs and executed as PE matmuls.

Key device-side structure (v2):
  * mm1 makes the channel-image chunk the STATIONARY operand:
        UT[x, yo] = sum_y S[y, x] Wy[yo, y]
    which yields U^T directly in PSUM -- no PE transpose, no extra copies.
  * mm2: F[yo, xo] = sum_x UT[x, yo] WxT[x, xo], rhs = WxT chunks.
  * All matmul operands fp16 (fast weight load, half DMA bytes); PSUM
    accumulates fp32; output DMA'd as fp32.
  * Host precomputes pm = colors*alpha and ca = 1-alpha (fp16), so the
    over scan is `over = over*ca + pm` (3 muls + 1 fused add per plane)
    on ping-pong fp16 state buffers (keeps the scan off the warp path).
  * Per plane per pass: one PSUM->SBUF UT copy, one F copy, one output
    DMA covering all 3-4 channels with 3-4KB contiguous lines.

Sharding: core = (s, h) in 4 sources x 2 output-row-halves; compositing is
pointwise in pixels -> fully core-local, no collectives.

Channel order on device: 0=T, 1..3=src, 4..6=acc, 7..9=bro (host reorders
to the reference order T, acc, bro, src).
"""

import sys

import numpy as np

sys.path.insert(0, "/opt/trn_rl_repo")

P, S, H, W = 32, 4, 256, 256
NCORES = 8
NCH = 10

# device channel k -> reference channel: ref order [T, acc*3, bro*3, src*3]
_CH_PERM = [0, 4, 5, 6, 7, 8, 9, 1, 2, 3]


def _compute_sample_coords(mpi_planes, pose_tgt, intrins_src, intrins_tgt):
    """Exact reference math for sample coords, float64. -> ix, iy (P,S,H,W)."""
    Kinv = np.linalg.inv(intrins_tgt.astype(np.float64))
    gx, gy = np.meshgrid(
        np.arange(W, dtype=np.float64), np.arange(H, dtype=np.float64)
    )
    pix = np.stack([gx.ravel(), gy.ravel(), np.ones(H * W)])  # (3, HW)
    cam_dir = Kinv @ pix  # (3, HW)
    ix = np.empty((P, S, H, W))
    iy = np.empty((P, S, H, W))
    for s in range(S):
        K4 = np.zeros((4, 4))
        K4[:3, :3] = intrins_src[s].astype(np.float64)
        K4[3, 3] = 1.0
        proj = K4 @ pose_tgt[s].astype(np.float64)
        for p in range(P):
            cam = np.concatenate(
                [cam_dir * np.float64(mpi_planes[p]), np.ones((1, H * W))], 0
            )
            upc = proj @ cam
            z = upc[2] + 1e-10
            ix[p, s] = (upc[0] / z).reshape(H, W)
            iy[p, s] = (upc[1] / z).reshape(H, W)
    return ix, iy


def _bilinear_matrix(coord_1d, n_in):
    """1D resample matrix M[out, in] with reference tap/validity semantics."""
    n_out = coord_1d.shape[0]
    M = np.zeros((n_out, n_in), np.float64)
    c0 = np.floor(coord_1d)
    w1 = coord_1d - c0
    w0 = 1.0 - w1
    for o in range(n_out):
        i0 = int(c0[o])
        if 0 <= i0 <= n_in - 1:
            M[o, i0] += w0[o]
        if 0 <= i0 + 1 <= n_in - 1:
            M[o, i0 + 1] += w1[o]
    return M


def _reference_numpy(colors, alphas, imgs_src, mpi_planes, pose_tgt,
                     intrins_src, intrins_tgt):
    """Pure-numpy replica of the reference (generic fallback + self-test)."""
    Pn, Sn, Hh, Ww = alphas.shape
    ca = 1.0 - alphas
    pm = colors * alphas[..., None]
    overs = np.empty_like(pm)
    over = np.zeros_like(pm[0])
    for d in range(Pn):
        over = over * ca[d][..., None] + pm[d]
        overs[d] = over
    acc = overs[np.maximum(np.arange(Pn) - 2, 0)]
    bro = np.broadcast_to(overs[-1][None], (Pn, Sn, Hh, Ww, 3))
    rc = np.cumprod(ca[::-1], axis=0)[::-1]
    T = np.concatenate([rc[1:], np.ones_like(rc[:1])], axis=0)
    src = np.broadcast_to(imgs_src[None], (Pn, Sn, Hh, Ww, 3))
    stacked = np.concatenate([T[..., None], acc, bro, src], axis=-1)

    ix, iy = _compute_sample_coords(mpi_planes, pose_tgt, intrins_src,
                                    intrins_tgt)
    out = np.empty((Pn, Sn, NCH, Hh, Ww), np.float32)
    for p in range(Pn):
        for s in range(Sn):
            img = stacked[p, s]
            x0 = np.floor(ix[p, s])
            y0 = np.floor(iy[p, s])
            wx1 = ix[p, s] - x0
            wx0 = 1.0 - wx1
            wy1 = iy[p, s] - y0
            wy0 = 1.0 - wy1

            def gather(xx, yy):
                valid = (xx >= 0) & (xx <= Ww - 1) & (yy >= 0) & (yy <= Hh - 1)
                xc = np.clip(xx, 0, Ww - 1).astype(np.int64)
                yc = np.clip(yy, 0, Hh - 1).astype(np.int64)
                return img[yc, xc] * valid[..., None]

            warped = (gather(x0, y0) * (wx0 * wy0)[..., None]
                      + gather(x0 + 1, y0) * (wx1 * wy0)[..., None]
                      + gather(x0, y0 + 1) * (wx0 * wy1)[..., None]
                      + gather(x0 + 1, y0 + 1) * (wx1 * wy1)[..., None])
            out[p, s] = warped.transpose(2, 0, 1).astype(np.float32)
    return out


_CACHED = {}


def _build_bass_program():
    """Build (once) the SPMD Bass program shared by all 8 cores."""
    if "nc" in _CACHED:
        return _CACHED["nc"]

    import concourse.bacc as bacc
    import concourse.mybir as mybir
    from concourse import tile

    f32 = mybir.dt.float32
    f16 = mybir.dt.float16

    nc = bacc.Bacc(
        "TRN2", target_bir_lowering=False, debug=False,
        enable_asserts=False, num_devices=NCORES,
    )

    ca_d = nc.dram_tensor("ca", [128, P, 2, W], f16, kind="ExternalInput").ap()
    wy_d = nc.dram_tensor("wy", [128, P, 2, 128], f16, kind="ExternalInput").ap()
    wx_d = nc.dram_tensor("wx", [128, P, 2, W], f16, kind="ExternalInput").ap()
    src_d = nc.dram_tensor("src", [128, 3, 2, W], f16, kind="ExternalInput").ap()
    pm_d = nc.dram_tensor("pm", [P, 128, 3, 2, W], f16, kind="ExternalInput").ap()
    out_d = nc.dram_tensor("out", [P, 128, NCH, W], f16,
                           kind="ExternalOutput").ap()

    with tile.TileContext(nc) as tc:
        with (
            tc.tile_pool(name="persist", bufs=1) as persist,
            tc.tile_pool(name="pm", bufs=6) as pm_pool,
            tc.tile_pool(name="work", bufs=4) as work,
            tc.tile_pool(name="psum", bufs=2, space="PSUM") as psum,
        ):
            wy_sb = persist.tile([128, P, 2, 128], f16, tag="wy", name="wy_sb")
            src_sb = persist.tile([128, 3, 2, W], f16, tag="src", name="src_sb")
            ca_sb = persist.tile([128, P, 2, W], f16, tag="ca", name="ca_sb")
            wx_sb = persist.tile([128, P, 2, W], f16, tag="wx", name="wx_sb")
            t16a = persist.tile([128, 2, W], f16, tag="t16a", name="t16a")
            t16b = persist.tile([128, 2, W], f16, tag="t16b", name="t16b")
            tbuf = [t16a, t16b]
            over0 = persist.tile([128, 3, 2, W], f16, tag="over0", name="over0")
            over1 = persist.tile([128, 3, 2, W], f16, tag="over1", name="over1")
            over = [over0, over1]

            # Input preload on the SCALAR (Activation) HWDGE ring so the
            # bulk input bytes don't head-of-line-block pm/output DMAs,
            # which ride the sync ring.  Bulk inputs are trickled in small
            # just-in-time chunks (rather than upfront megabyte blasts) so
            # the 16 SDMA engines always have capacity for output DMAs —
            # otherwise the fout pool fills and compute stalls at startup.
            # Pass A consumes planes descending (31->0); pass B's warps
            # need planes ascending from 3; chunks serve both ends.
            nc.scalar.dma_start(wy_sb[:, 28:32], wy_d[:, 28:32])
            nc.scalar.dma_start(wx_sb[:, 28:32], wx_d[:, 28:32])
            nc.scalar.dma_start(src_sb[:], src_d[:])
            nc.scalar.dma_start(ca_sb[:, 24:32], ca_d[:, 24:32])
            nc.sync.dma_start(over[0][:], pm_d[0])
            nc.scalar.dma_start(ca_sb[:, 0:8], ca_d[:, 0:8])
            nc.scalar.dma_start(wy_sb[:, 0:6], wy_d[:, 0:6])
            nc.scalar.dma_start(wx_sb[:, 0:6], wx_d[:, 0:6])
            nc.gpsimd.memset(t16a[:], 1.0)

            _in_chunks = {
                1: [(wy_sb, wy_d, 24, 28), (wx_sb, wx_d, 24, 28)],
                2: [(wy_sb, wy_d, 6, 12), (wx_sb, wx_d, 6, 12)],
                3: [(wy_sb, wy_d, 20, 24), (wx_sb, wx_d, 20, 24)],
                4: [(ca_sb, ca_d, 8, 24)],
                5: [(wy_sb, wy_d, 12, 20), (wx_sb, wx_d, 12, 20)],
            }

            # pm prefetch: keep DMAs ~4 planes ahead of the scan.
            pm_tiles = {}

            def issue_pm(d):
                if 1 <= d < P:
                    t = pm_pool.tile([128, 3, 2, W], f16, tag="pm",
                                     name="pm_t")
                    nc.sync.dma_start(t[:], pm_d[d])
                    pm_tiles[d] = t

            for d in (1, 2, 3):
                issue_pm(d)

            def _copy(eng, out, in_):
                if eng is nc.scalar:
                    eng.copy(out, in_)
                else:
                    eng.tensor_copy(out, in_)

            def warp_batch(d, imgs, ch0, ut_eng, f_eng):
                """Warp len(imgs) channel images with plane-d matrices ->
                out[d, :, ch0:ch0+k, :].  Each img: AP [128, 2, W]."""
                k = len(imgs)
                ut_ps = psum.tile([128, 1024], f32, tag="ut", name="ut_ps")
                for i, img in enumerate(imgs):
                    for cx in (0, 1):
                        co = i * 256 + cx * 128
                        for cy in (0, 1):
                            nc.tensor.matmul(
                                ut_ps[:, co:co + 128],
                                img[:, cy, cx * 128:(cx + 1) * 128],
                                wy_sb[:, d, cy, :],
                                start=(cy == 0), stop=(cy == 1),
                            )
                ut16 = work.tile([128, 1024], f16, tag="ut16", name="ut16")
                _copy(ut_eng, ut16[:, 0:k * 256], ut_ps[:, 0:k * 256])
                f_ps = psum.tile([128, 1024], f32, tag="f", name="f_ps")
                for i in range(k):
                    for cx in (0, 1):
                        nc.tensor.matmul(
                            f_ps[:, i * 256:(i + 1) * 256],
                            ut16[:, i * 256 + cx * 128:i * 256 + cx * 128 + 128],
                            wx_sb[:, d, cx, :],
                            start=(cx == 0), stop=(cx == 1),
                        )
                fout = work.tile([128, 1024], f16, tag="fout", name="fout")
                _copy(f_eng, fout[:, 0:k * 256], f_ps[:, 0:k * 256])
                nc.sync.dma_start(
                    out_d[d, :, ch0:ch0 + k, :],
                    fout[:, 0:k * 256].rearrange("p (c w) -> p c w", c=k),
                )

            # Interleave pass A (backward T scan; T + src channels) with
            # pass B (forward over scan; acc channels) for scheduler overlap.
            for i in range(P):
                for (sb, dr, lo, hi) in _in_chunks.get(i, ()):
                    nc.scalar.dma_start(sb[:, lo:hi], dr[:, lo:hi])
                da = P - 1 - i
                # ---- pass A step: warp T + src for plane da, then T *= ca
                told, tnew = tbuf[i % 2], tbuf[(i + 1) % 2]
                warp_batch(da, [told[:], src_sb[:, 0], src_sb[:, 1],
                                src_sb[:, 2]], 0, nc.scalar, nc.scalar)
                if da > 0:
                    nc.gpsimd.tensor_mul(tnew[:], told[:], ca_sb[:, da])

                # ---- pass B step: over scan at plane i; warp acc
                db = i
                issue_pm(db + 4)
                if db == 0:
                    for pl in (0, 1, 2):
                        warp_batch(pl, [over[0][:, 0], over[0][:, 1],
                                        over[0][:, 2]], 4,
                                   nc.vector, nc.scalar)
                else:
                    pm_t = pm_tiles.pop(db)
                    prev, cur = over[(db + 1) % 2], over[db % 2]
                    for c in range(2):
                        nc.vector.tensor_mul(cur[:, c], prev[:, c],
                                             ca_sb[:, db])
                    nc.gpsimd.tensor_mul(cur[:, 2], prev[:, 2], ca_sb[:, db])
                    nc.vector.tensor_add(cur[:], cur[:], pm_t[:])
                    if db <= P - 3:
                        warp_batch(db + 2, [cur[:, 0], cur[:, 1],
                                            cur[:, 2]], 4,
                                   nc.vector, nc.scalar)

            # ---- pass C: warp bro = overs[-1] for every plane ------------
            final = over[(P - 1) % 2]
            for d in range(P):
                warp_batch(d, [final[:, 0], final[:, 1], final[:, 2]], 7,
                           nc.scalar, nc.vector)

    nc.compile()
    _CACHED["nc"] = nc
    return nc


def _host_prepare(colors, alphas, imgs_src, mpi_planes, pose_tgt,
                  intrins_src, intrins_tgt):
    """Build per-core input maps. Returns (in_maps, separable)."""
    ix, iy = _compute_sample_coords(mpi_planes, pose_tgt, intrins_src,
                                    intrins_tgt)
    dev_x = np.abs(ix - ix[:, :, :1, :]).max()
    dev_y = np.abs(iy - iy[:, :, :, :1]).max()
    if dev_x > 1e-3 or dev_y > 1e-3:
        return None, False

    ix1 = ix[:, :, 0, :]  # (P, S, W)
    iy1 = iy[:, :, :, 0]  # (P, S, H)

    in_maps = [None] * NCORES
    for s in range(S):
        a = alphas[:, s]  # (P, H, W)
        ca = (1.0 - a).astype(np.float16)
        ca_r = np.ascontiguousarray(
            ca.reshape(P, 2, 128, W).transpose(2, 0, 1, 3))  # (128,P,2,W)
        pm = (colors[:, s] * a[..., None]).astype(np.float16)  # (P,H,W,3)
        pm_r = np.ascontiguousarray(
            pm.reshape(P, 2, 128, W, 3).transpose(0, 2, 4, 1, 3))
        src_r = np.ascontiguousarray(
            imgs_src[s].astype(np.float16)
            .reshape(2, 128, W, 3).transpose(1, 3, 0, 2))  # (128,3,2,W)

        # x-resample matrices are shared by both h-halves of this source
        wx = np.empty((P, W, W), np.float64)
        for d in range(P):
            wx[d] = _bilinear_matrix(ix1[d, s], W).T  # [xi, xo]
        wx_r = np.ascontiguousarray(
            wx.reshape(P, 2, 128, W).transpose(2, 0, 1, 3)).astype(np.float16)

        for h in range(2):
            wy = np.empty((P, H, 128), np.float64)
            for d in range(P):
                wy[d] = _bilinear_matrix(
                    iy1[d, s, h * 128:(h + 1) * 128], H).T  # [yi, yo]
            wy_r = np.ascontiguousarray(
                wy.reshape(P, 2, 128, 128).transpose(2, 0, 1, 3)
            ).astype(np.float16)
            in_maps[s * 2 + h] = {
                "ca": ca_r, "wy": wy_r, "wx": wx_r,
                "src": src_r, "pm": pm_r,
            }
    return in_maps, True


def kernel(colors, alphas, imgs_src, mpi_planes, pose_tgt, intrins_src,
           intrins_tgt):
    colors = np.asarray(colors, np.float32)
    alphas = np.asarray(alphas, np.float32)
    imgs_src = np.asarray(imgs_src, np.float32)
    mpi_planes = np.asarray(mpi_planes, np.float32)
    pose_tgt = np.asarray(pose_tgt, np.float32)
    intrins_src = np.asarray(intrins_src, np.float32)
    intrins_tgt = np.asarray(intrins_tgt, np.float32)

    in_maps, separable = _host_prepare(
        colors, alphas, imgs_src, mpi_planes, pose_tgt, intrins_src,
        intrins_tgt)
    if not separable:
        return _reference_numpy(colors, alphas, imgs_src, mpi_planes,
                                pose_tgt, intrins_src, intrins_tgt)

    from concourse.bass_utils import run_bass_kernel_spmd

    nc = _build_bass_program()
    res = run_bass_kernel_spmd(nc, in_maps, core_ids=list(range(NCORES)))
    _CACHED["last_results"] = res

    out = np.empty((P, S, NCH, H, W), np.float32)
    for core in range(NCORES):
        s, h = divmod(core, 2)
        dev = res.results[core]["out"]  # (P, 128, NCH, W) f16
        out[:, s, :, h * 128:(h + 1) * 128, :] = \
            dev.transpose(0, 2, 1, 3)[:, _CH_PERM].astype(np.float32)
    return out


# revision 23
# speedup vs baseline: 1.1249x; 1.1249x over previous
"""MPI compositing + homography warp kernel for Trainium2 (8 NeuronCores).

For each of P=32 fronto-parallel planes and S=4 source images: composite
per-plane channels (net transmittance T, accumulated-over acc, full-over
bro, source image src -> 10 channels), then bilinear-warp each (plane, src)
channel stack by a plane/source-dependent homography. Output (P, S, 10, H, W).

Structure exploited: the target->source homography here has identity
rotation and shared intrinsics, so sample coordinate ix depends only on x
and iy only on y.  The bilinear gather (zero padding) then factorizes
EXACTLY into two small banded matrices applied left/right:

    warped = Wy @ S @ Wx^T        per (plane, src, channel)

with per-tap validity folded into the weights.  Wy/Wx are built on the host
from the pose inputs and executed as PE matmuls.

Key device-side structure (v2):
  * mm1 makes the channel-image chunk the STATIONARY operand:
        UT[x, yo] = sum_y S[y, x] Wy[yo, y]
    which yields U^T directly in PSUM -- no PE transpose, no extra copies.
  * mm2: F[yo, xo] = sum_x UT[x, yo] WxT[x, xo], rhs = WxT chunks.
  * All matmul operands fp16 (fast weight load, half DMA bytes); PSUM
    accumulates fp32; output DMA'd as fp32.
  * Host precomputes pm = colors*alpha and ca = 1-alpha (fp16), so the
    over scan is `over = over*ca + pm` (3 muls + 1 fused add per plane)
    on ping-pong fp16 state buffers (keeps the scan off the warp path).
  * Per plane per pass: one PSUM->SBUF UT copy, one F copy, one output
    DMA covering all 3-4 channels with 3-4KB contiguous lines.

Sharding: core = (s, h) in 4 sources x 2 output-row-halves; compositing is
pointwise in pixels -> fully core-local, no collectives.

Channel order on device: 0=T, 1..3=src, 4..6=acc, 7..9=bro (host reorders
to the reference order T, acc, bro, src).
"""

import sys

import numpy as np

sys.path.insert(0, "/opt/trn_rl_repo")

P, S, H, W = 32, 4, 256, 256
NCORES = 8
NCH = 10

# device channel k -> reference channel: ref order [T, acc*3, bro*3, src*3]
_CH_PERM = [0, 4, 5, 6, 7, 8, 9, 1, 2, 3]


def _compute_sample_coords(mpi_planes, pose_tgt, intrins_src, intrins_tgt):
    """Exact reference math for sample coords, float64. -> ix, iy (P,S,H,W)."""
    Kinv = np.linalg.inv(intrins_tgt.astype(np.float64))
    gx, gy = np.meshgrid(
        np.arange(W, dtype=np.float64), np.arange(H, dtype=np.float64)
    )
    pix = np.stack([gx.ravel(), gy.ravel(), np.ones(H * W)])  # (3, HW)
    cam_dir = Kinv @ pix  # (3, HW)
    ix = np.empty((P, S, H, W))
    iy = np.empty((P, S, H, W))
    for s in range(S):
        K4 = np.zeros((4, 4))
        K4[:3, :3] = intrins_src[s].astype(np.float64)
        K4[3, 3] = 1.0
        proj = K4 @ pose_tgt[s].astype(np.float64)
        for p in range(P):
            cam = np.concatenate(
                [cam_dir * np.float64(mpi_planes[p]), np.ones((1, H * W))], 0
            )
            upc = proj @ cam
            z = upc[2] + 1e-10
            ix[p, s] = (upc[0] / z).reshape(H, W)
            iy[p, s] = (upc[1] / z).reshape(H, W)
    return ix, iy


def _bilinear_matrix(coord_1d, n_in):
    """1D resample matrix M[out, in] with reference tap/validity semantics."""
    n_out = coord_1d.shape[0]
    M = np.zeros((n_out, n_in), np.float64)
    c0 = np.floor(coord_1d)
    w1 = coord_1d - c0
    w0 = 1.0 - w1
    for o in range(n_out):
        i0 = int(c0[o])
        if 0 <= i0 <= n_in - 1:
            M[o, i0] += w0[o]
        if 0 <= i0 + 1 <= n_in - 1:
            M[o, i0 + 1] += w1[o]
    return M


def _reference_numpy(colors, alphas, imgs_src, mpi_planes, pose_tgt,
                     intrins_src, intrins_tgt):
    """Pure-numpy replica of the reference (generic fallback + self-test)."""
    Pn, Sn, Hh, Ww = alphas.shape
    ca = 1.0 - alphas
    pm = colors * alphas[..., None]
    overs = np.empty_like(pm)
    over = np.zeros_like(pm[0])
    for d in range(Pn):
        over = over * ca[d][..., None] + pm[d]
        overs[d] = over
    acc = overs[np.maximum(np.arange(Pn) - 2, 0)]
    bro = np.broadcast_to(overs[-1][None], (Pn, Sn, Hh, Ww, 3))
    rc = np.cumprod(ca[::-1], axis=0)[::-1]
    T = np.concatenate([rc[1:], np.ones_like(rc[:1])], axis=0)
    src = np.broadcast_to(imgs_src[None], (Pn, Sn, Hh, Ww, 3))
    stacked = np.concatenate([T[..., None], acc, bro, src], axis=-1)

    ix, iy = _compute_sample_coords(mpi_planes, pose_tgt, intrins_src,
                                    intrins_tgt)
    out = np.empty((Pn, Sn, NCH, Hh, Ww), np.float32)
    for p in range(Pn):
        for s in range(Sn):
            img = stacked[p, s]
            x0 = np.floor(ix[p, s])
            y0 = np.floor(iy[p, s])
            wx1 = ix[p, s] - x0
            wx0 = 1.0 - wx1
            wy1 = iy[p, s] - y0
            wy0 = 1.0 - wy1

            def gather(xx, yy):
                valid = (xx >= 0) & (xx <= Ww - 1) & (yy >= 0) & (yy <= Hh - 1)
                xc = np.clip(xx, 0, Ww - 1).astype(np.int64)
                yc = np.clip(yy, 0, Hh - 1).astype(np.int64)
                return img[yc, xc] * valid[..., None]

            warped = (gather(x0, y0) * (wx0 * wy0)[..., None]
                      + gather(x0 + 1, y0) * (wx1 * wy0)[..., None]
                      + gather(x0, y0 + 1) * (wx0 * wy1)[..., None]
                      + gather(x0 + 1, y0 + 1) * (wx1 * wy1)[..., None])
            out[p, s] = warped.transpose(2, 0, 1).astype(np.float32)
    return out


_CACHED = {}


def _build_bass_program():
    """Build (once) the SPMD Bass program shared by all 8 cores."""
    if "nc" in _CACHED:
        return _CACHED["nc"]

    import concourse.bacc as bacc
    import concourse.mybir as mybir
    from concourse import tile

    f32 = mybir.dt.float32
    f16 = mybir.dt.float16

    nc = bacc.Bacc(
        "TRN2", target_bir_lowering=False, debug=False,
        enable_asserts=False, num_devices=NCORES,
    )

    ca_d = nc.dram_tensor("ca", [128, P, 2, W], f16, kind="ExternalInput").ap()
    wy_d = nc.dram_tensor("wy", [128, P, 2, 128], f16, kind="ExternalInput").ap()
    wx_d = nc.dram_tensor("wx", [128, P, 2, W], f16, kind="ExternalInput").ap()
    src_d = nc.dram_tensor("src", [128, 3, 2, W], f16, kind="ExternalInput").ap()
    pm_d = nc.dram_tensor("pm", [P, 128, 3, 2, W], f16, kind="ExternalInput").ap()
    out_d = nc.dram_tensor("out", [P, 128, NCH, W], f16,
                           kind="ExternalOutput").ap()

    with tile.TileContext(nc) as tc:
        with (
            tc.tile_pool(name="persist", bufs=1) as persist,
            tc.tile_pool(name="pm", bufs=6) as pm_pool,
            tc.tile_pool(name="work", bufs=4) as work,
            tc.tile_pool(name="psum", bufs=2, space="PSUM") as psum,
        ):
            wy_sb = persist.tile([128, P, 2, 128], f16, tag="wy", name="wy_sb")
            src_sb = persist.tile([128, 3, 2, W], f16, tag="src", name="src_sb")
            ca_sb = persist.tile([128, P, 2, W], f16, tag="ca", name="ca_sb")
            wx_sb = persist.tile([128, P, 2, W], f16, tag="wx", name="wx_sb")
            t16a = persist.tile([128, 2, W], f16, tag="t16a", name="t16a")
            t16b = persist.tile([128, 2, W], f16, tag="t16b", name="t16b")
            tbuf = [t16a, t16b]
            over0 = persist.tile([128, 3, 2, W], f16, tag="over0", name="over0")
            over1 = persist.tile([128, 3, 2, W], f16, tag="over1", name="over1")
            over = [over0, over1]

            # Input preload on the SCALAR (Activation) HWDGE ring so the
            # bulk input bytes don't head-of-line-block pm/output DMAs,
            # which ride the sync ring.  Bulk inputs are trickled in small
            # just-in-time chunks (rather than upfront megabyte blasts) so
            # the 16 SDMA engines always have capacity for output DMAs —
            # otherwise the fout pool fills and compute stalls at startup.
            # Pass A consumes planes descending (31->0); pass B's warps
            # need planes ascending from 3; chunks serve both ends.
            nc.scalar.dma_start(wy_sb[:, 30:32], wy_d[:, 30:32])
            nc.scalar.dma_start(src_sb[:], src_d[:])
            nc.scalar.dma_start(wx_sb[:, 30:32], wx_d[:, 30:32])
            nc.scalar.dma_start(ca_sb[:, 24:32], ca_d[:, 24:32])
            nc.sync.dma_start(over[0][:], pm_d[0])
            nc.sync.dma_start(ca_sb[:, 0:8], ca_d[:, 0:8])
            nc.scalar.dma_start(wy_sb[:, 24:30], wy_d[:, 24:30])
            nc.scalar.dma_start(wx_sb[:, 24:30], wx_d[:, 24:30])
            nc.scalar.dma_start(wy_sb[:, 0:6], wy_d[:, 0:6])
            nc.scalar.dma_start(wx_sb[:, 0:6], wx_d[:, 0:6])
            nc.gpsimd.memset(t16a[:], 1.0)

            # PE pre-warm: ~5us of dummy matmuls during the input preload
            # flips the HAM clock gate to 8/8 (2.4 GHz) before real work.
            zwarm = persist.tile([128, 128], f16, tag="zwarm", name="zwarm")
            nc.gpsimd.memset(zwarm[:], 0.0)
            warm_ps = psum.tile([128, 1024], f32, tag="ut", name="warm_ps")
            for _ in range(24):
                nc.tensor.matmul(warm_ps[:, 0:128], zwarm[:], zwarm[:],
                                 start=True, stop=True)

            _in_chunks = {
                1: [(wy_sb, wy_d, 18, 24), (wx_sb, wx_d, 18, 24)],
                2: [(wy_sb, wy_d, 6, 12), (wx_sb, wx_d, 6, 12)],
                3: [(ca_sb, ca_d, 8, 24)],
                4: [(wy_sb, wy_d, 12, 18), (wx_sb, wx_d, 12, 18)],
            }

            # pm prefetch: keep DMAs ~4 planes ahead of the scan.
            pm_tiles = {}

            def issue_pm(d):
                if 1 <= d < P:
                    t = pm_pool.tile([128, 3, 2, W], f16, tag="pm",
                                     name="pm_t")
                    nc.sync.dma_start(t[:], pm_d[d])
                    pm_tiles[d] = t

            for d in (1, 2, 3):
                issue_pm(d)

            def _copy(eng, out, in_):
                if eng is nc.scalar:
                    eng.copy(out, in_)
                else:
                    eng.tensor_copy(out, in_)

            def warp_batch(d, imgs, ch0, ut_eng, f_eng):
                """Warp len(imgs) channel images with plane-d matrices ->
                out[d, :, ch0:ch0+k, :].  Each img: AP [128, 2, W]."""
                k = len(imgs)
                ut_ps = psum.tile([128, 1024], f32, tag="ut", name="ut_ps")
                for i, img in enumerate(imgs):
                    for cx in (0, 1):
                        co = i * 256 + cx * 128
                        for cy in (0, 1):
                            nc.tensor.matmul(
                                ut_ps[:, co:co + 128],
                                img[:, cy, cx * 128:(cx + 1) * 128],
                                wy_sb[:, d, cy, :],
                                start=(cy == 0), stop=(cy == 1),
                            )
                ut16 = work.tile([128, 1024], f16, tag="ut16", name="ut16")
                _copy(ut_eng, ut16[:, 0:k * 256], ut_ps[:, 0:k * 256])
                f_ps = psum.tile([128, 1024], f32, tag="f", name="f_ps")
                for i in range(k):
                    for cx in (0, 1):
                        nc.tensor.matmul(
                            f_ps[:, i * 256:(i + 1) * 256],
                            ut16[:, i * 256 + cx * 128:i * 256 + cx * 128 + 128],
                            wx_sb[:, d, cx, :],
                            start=(cx == 0), stop=(cx == 1),
                        )
                fout = work.tile([128, 1024], f16, tag="fout", name="fout")
                _copy(f_eng, fout[:, 0:k * 256], f_ps[:, 0:k * 256])
                nc.sync.dma_start(
                    out_d[d, :, ch0:ch0 + k, :],
                    fout[:, 0:k * 256].rearrange("p (c w) -> p c w", c=k),
                )

            # Interleave pass A (backward T scan; T + src channels) with
            # pass B (forward over scan; acc channels) for scheduler overlap.
            for i in range(P):
                for (sb, dr, lo, hi) in _in_chunks.get(i, ()):
                    nc.scalar.dma_start(sb[:, lo:hi], dr[:, lo:hi])
                da = P - 1 - i
                # ---- pass A step: warp T + src for plane da, then T *= ca
                told, tnew = tbuf[i % 2], tbuf[(i + 1) % 2]
                warp_batch(da, [told[:], src_sb[:, 0], src_sb[:, 1],
                                src_sb[:, 2]], 0, nc.scalar, nc.vector)
                if da > 0:
                    nc.gpsimd.tensor_mul(tnew[:], told[:], ca_sb[:, da])

                # ---- pass B step: over scan at plane i; warp acc
                db = i
                issue_pm(db + 4)
                if db == 0:
                    for pl in (0, 1, 2):
                        warp_batch(pl, [over[0][:, 0], over[0][:, 1],
                                        over[0][:, 2]], 4,
                                   nc.scalar, nc.scalar)
                else:
                    pm_t = pm_tiles.pop(db)
                    prev, cur = over[(db + 1) % 2], over[db % 2]
                    for c in range(2):
                        nc.vector.tensor_mul(cur[:, c], prev[:, c],
                                             ca_sb[:, db])
                    nc.gpsimd.tensor_mul(cur[:, 2], prev[:, 2], ca_sb[:, db])
                    nc.vector.tensor_add(cur[:], cur[:], pm_t[:])
                    if db <= P - 3:
                        warp_batch(db + 2, [cur[:, 0], cur[:, 1],
                                            cur[:, 2]], 4,
                                   nc.scalar, nc.scalar)

            # ---- pass C: warp bro = overs[-1] for every plane ------------
            final = over[(P - 1) % 2]
            for d in range(P):
                warp_batch(d, [final[:, 0], final[:, 1], final[:, 2]], 7,
                           nc.scalar, nc.vector)

    nc.compile()
    _CACHED["nc"] = nc
    return nc


def _host_prepare(colors, alphas, imgs_src, mpi_planes, pose_tgt,
                  intrins_src, intrins_tgt):
    """Build per-core input maps. Returns (in_maps, separable)."""
    ix, iy = _compute_sample_coords(mpi_planes, pose_tgt, intrins_src,
                                    intrins_tgt)
    dev_x = np.abs(ix - ix[:, :, :1, :]).max()
    dev_y = np.abs(iy - iy[:, :, :, :1]).max()
    if dev_x > 1e-3 or dev_y > 1e-3:
        return None, False

    ix1 = ix[:, :, 0, :]  # (P, S, W)
    iy1 = iy[:, :, :, 0]  # (P, S, H)

    in_maps = [None] * NCORES
    for s in range(S):
        a = alphas[:, s]  # (P, H, W)
        ca = (1.0 - a).astype(np.float16)
        ca_r = np.ascontiguousarray(
            ca.reshape(P, 2, 128, W).transpose(2, 0, 1, 3))  # (128,P,2,W)
        pm = (colors[:, s] * a[..., None]).astype(np.float16)  # (P,H,W,3)
        pm_r = np.ascontiguousarray(
            pm.reshape(P, 2, 128, W, 3).transpose(0, 2, 4, 1, 3))
        src_r = np.ascontiguousarray(
            imgs_src[s].astype(np.float16)
            .reshape(2, 128, W, 3).transpose(1, 3, 0, 2))  # (128,3,2,W)

        # x-resample matrices are shared by both h-halves of this source
        wx = np.empty((P, W, W), np.float64)
        for d in range(P):
            wx[d] = _bilinear_matrix(ix1[d, s], W).T  # [xi, xo]
        wx_r = np.ascontiguousarray(
            wx.reshape(P, 2, 128, W).transpose(2, 0, 1, 3)).astype(np.float16)

        for h in range(2):
            wy = np.empty((P, H, 128), np.float64)
            for d in range(P):
                wy[d] = _bilinear_matrix(
                    iy1[d, s, h * 128:(h + 1) * 128], H).T  # [yi, yo]
            wy_r = np.ascontiguousarray(
                wy.reshape(P, 2, 128, 128).transpose(2, 0, 1, 3)
            ).astype(np.float16)
            in_maps[s * 2 + h] = {
                "ca": ca_r, "wy": wy_r, "wx": wx_r,
                "src": src_r, "pm": pm_r,
            }
    return in_maps, True


def kernel(colors, alphas, imgs_src, mpi_planes, pose_tgt, intrins_src,
           intrins_tgt):
    colors = np.asarray(colors, np.float32)
    alphas = np.asarray(alphas, np.float32)
    imgs_src = np.asarray(imgs_src, np.float32)
    mpi_planes = np.asarray(mpi_planes, np.float32)
    pose_tgt = np.asarray(pose_tgt, np.float32)
    intrins_src = np.asarray(intrins_src, np.float32)
    intrins_tgt = np.asarray(intrins_tgt, np.float32)

    in_maps, separable = _host_prepare(
        colors, alphas, imgs_src, mpi_planes, pose_tgt, intrins_src,
        intrins_tgt)
    if not separable:
        return _reference_numpy(colors, alphas, imgs_src, mpi_planes,
                                pose_tgt, intrins_src, intrins_tgt)

    from concourse.bass_utils import run_bass_kernel_spmd

    nc = _build_bass_program()
    res = run_bass_kernel_spmd(nc, in_maps, core_ids=list(range(NCORES)))
    _CACHED["last_results"] = res

    out = np.empty((P, S, NCH, H, W), np.float32)
    for core in range(NCORES):
        s, h = divmod(core, 2)
        dev = res.results[core]["out"]  # (P, 128, NCH, W) f16
        out[:, s, :, h * 128:(h + 1) * 128, :] = \
            dev.transpose(0, 2, 1, 3)[:, _CH_PERM].astype(np.float32)
    return out
